# revision 1
# baseline (speedup 1.0000x reference)
"""GAT (2-layer, PyG-style) on 8 Trainium2 NeuronCores via Bass/Tile.

Strategy (dst-sharded, degree-sorted tiles):
- Nodes sharded by dst across 8 cores (12500 each). Per core, dsts are
  degree-sorted and grouped into 98 tiles of 128 (partition = dst).
- Per tile, column 0 gathers the dst's own table row (serving both the
  self-loop edge and the per-partition al_dst values); remaining columns
  hold in-edges, padded to the tile max degree with masked slots.
- Layer tables are 4-node-packed rows (<=32767 rows, int16 dma_gather
  indices); a 4-way one-hot select on DVE picks the node within the row.
- Segment softmax: no max-subtraction needed (logits are O(1)); the
  denominator divides the aggregated numerator once per dst row.
- b1/b2 folded into the h-columns of the tables (alpha sums to 1).
- Layer-2 per-node features (4 values) are exchanged via an on-chip
  AllGather of 4-packed shards in core-local permuted order.
"""

import math
import sys
import types

import numpy as np
import ml_dtypes

BF16 = ml_dtypes.bfloat16

N = 100_000
E = 3_200_000
IN = 128
H1, C1 = 8, 8
HID = H1 * C1          # 64
OUT = 2
NEG = 0.2
NCORES = 8
ND = N // NCORES       # dsts per core: 12500
NT = 98                # tiles per core (98*128 = 12544)
PT = NT * 128          # padded dst slots per core
NPAD = 100_352         # x padded to 784*128 (and divisible by 4)
T1ROWS = NPAD // 4     # 25088 4-pack rows, row = 4*(8+64+8) vals pad to 384
T1W = 384              # bf16 elems per table1 row (768B)
T2ROWS = (PT * NCORES) // 4   # 25088
T2W = 128              # bf16 elems per table2 row (256B); 16 used
NI = 1024              # idxs per dma_gather call (8 columns)
CPC = 8                # columns per call


def _wrap_idx(flat):
    """int16 index array -> [128, n/16] wrapped-in-16-partitions, replicated x8."""
    n = flat.shape[0]
    assert n % 16 == 0
    w = flat.reshape(n // 16, 16).T            # [16, n/16]
    return np.tile(w, (8, 1)).astype(np.int16)  # [128, n/16]


def _plan(src, dst):
    """Host-side index planning. Returns per-core data arrays + common schedule."""
    core = dst // ND
    dloc = dst % ND

    per_core = []
    for c in range(NCORES):
        m = core == c
        s_c = src[m]
        d_c = dloc[m]
        deg = np.bincount(d_c, minlength=ND)  # in-edges, no self loop yet
        order = np.argsort(-deg, kind="stable")  # degree-desc permutation
        perm = np.full(PT, -1, dtype=np.int64)
        perm[:ND] = order
        degp = np.zeros(PT, dtype=np.int64)
        degp[:ND] = deg[order]
        # group in-edges by dst for slot filling
        sort_by_d = np.argsort(d_c, kind="stable")
        s_sorted = s_c[sort_by_d]
        starts = np.zeros(ND + 1, dtype=np.int64)
        np.cumsum(deg, out=starts[1:])
        per_core.append(dict(perm=perm, degp=degp, s_sorted=s_sorted, starts=starts))

    # common K_t schedule: columns per tile = 1 (self/dst col) + max in-degree
    K = np.zeros(NT, dtype=np.int64)
    for t in range(NT):
        mx = 0
        for c in range(NCORES):
            d = per_core[c]["degp"][t * 128 : (t + 1) * 128]
            mx = max(mx, int(d.max()) if d.size else 0)
        K[t] = mx + 1
    ncols = int(K.sum())
    ncalls = (ncols + CPC - 1) // CPC
    ncols_pad = ncalls * CPC

    ctile = np.zeros(ncols_pad, dtype=np.int64) - 1   # column -> tile
    cj = np.zeros(ncols_pad, dtype=np.int64)          # column -> j within tile
    col0 = np.zeros(NT, dtype=np.int64)               # tile -> first column
    pos = 0
    for t in range(NT):
        col0[t] = pos
        ctile[pos : pos + K[t]] = t
        cj[pos : pos + K[t]] = np.arange(K[t])
        pos += K[t]

    # per-core slot arrays
    datas = []
    for c in range(NCORES):
        pc = per_core[c]
        perm, degp, s_sorted, starts = (
            pc["perm"], pc["degp"], pc["s_sorted"], pc["starts"],
        )
        node1 = np.zeros((ncols_pad, 128), dtype=np.int64)   # global node id (L1)
        valid = np.zeros((ncols_pad, 128), dtype=bool)
        for t in range(NT):
            base = col0[t]
            d_orig = perm[t * 128 : (t + 1) * 128]           # local dst ids, -1 pad
            real = d_orig >= 0
            dg = np.where(real, d_orig, 0)
            # column 0: the dst's own row (self loop + al_dst source)
            node1[base, :] = c * ND + dg
            valid[base, :] = real
            # in-edge columns
            kt = K[t]
            if kt > 1:
                st = starts[dg]
                cnt = degp[t * 128 : (t + 1) * 128]
                for j in range(1, kt):
                    sel = (j - 1 < cnt) & real
                    idxs = st + (j - 1)
                    node1[base + j, sel] = s_sorted[np.where(sel, idxs, 0)][sel]
                    valid[base + j, sel] = True
        datas.append(dict(node1=node1, valid=valid, perm=pc["perm"]))
    return datas, K, col0, ctile, cj, ncalls, ncols_pad


def _pack_inputs(datas, gpos_of_node, chunks, ncols_pad):
    """Build per-core device input arrays from the slot plan."""
    per_core_inputs = []
    for c in range(NCORES):
        node1 = datas[c]["node1"]      # [ncols_pad, 128]
        valid = datas[c]["valid"]

        idx1_flat = np.where(valid, node1 // 4, 0).astype(np.int16)      # slot i = col*128+p
        oh1 = np.zeros((ncols_pad, 128, 4), dtype=BF16)
        sub = (node1 % 4).astype(np.int64)
        ohv = np.eye(4, dtype=np.float32)[sub] * valid[:, :, None]
        oh1[:] = ohv.astype(BF16)
        padm = np.where(valid, 0.0, -1e30).astype(BF16)                  # [ncols_pad, 128]

        g = gpos_of_node[node1]        # permuted global position (L2 table)
        idx2_flat = np.where(valid, g // 4, 0).astype(np.int16)
        oh2 = np.zeros((ncols_pad, 128, 4), dtype=BF16)
        ohv2 = np.eye(4, dtype=np.float32)[(g % 4).astype(np.int64)] * valid[:, :, None]
        oh2[:] = ohv2.astype(BF16)

        # wrap indices per gather chunk (per tile, <=8 columns each)
        w1l, w2l = [], []
        for (c0, nc_) in chunks:
            f1 = idx1_flat[c0 : c0 + nc_].reshape(-1)
            f2 = idx2_flat[c0 : c0 + nc_].reshape(-1)
            w1l.append(_wrap_idx(f1))
            w2l.append(_wrap_idx(f2))
        idx1_w = np.concatenate(w1l, axis=1)
        idx2_w = np.concatenate(w2l, axis=1)

        per_core_inputs.append(dict(
            idx1=idx1_w,
            idx2=idx2_w,
            oh1=np.ascontiguousarray(oh1.transpose(1, 0, 2)),
            oh2=np.ascontiguousarray(oh2.transpose(1, 0, 2)),
            padm=np.ascontiguousarray(padm.T),
        ))
    return per_core_inputs


_BUILD_CACHE = {}


def _build(K, col0, chunks, totc, ncols_pad, phases="all"):
    import os as _os
    import concourse.bass as bass
    import concourse.bacc as bacc
    import concourse.mybir as mybir
    import concourse.tile as tile
    from concourse.masks import make_identity

    f32 = mybir.dt.float32
    bf16 = mybir.dt.bfloat16
    i16 = mybir.dt.int16
    AX = mybir.AxisListType.X
    OP = mybir.AluOpType
    ACT = mybir.ActivationFunctionType

    nc = bacc.Bacc("TRN2", target_bir_lowering=False, debug=False,
                   num_devices=NCORES, num_swdge_queues=4)

    x = nc.dram_tensor("x", [NPAD, IN], f32, kind="ExternalInput")
    w1e = nc.dram_tensor("w1e", [IN, 80], f32, kind="ExternalInput")
    w2e = nc.dram_tensor("w2e", [HID, 4], bf16, kind="ExternalInput")
    idx1 = nc.dram_tensor("idx1", [128, totc], i16, kind="ExternalInput")
    idx2 = nc.dram_tensor("idx2", [128, totc], i16, kind="ExternalInput")
    oh1 = nc.dram_tensor("oh1", [128, ncols_pad, 4], bf16, kind="ExternalInput")
    oh2 = nc.dram_tensor("oh2", [128, ncols_pad, 4], bf16, kind="ExternalInput")
    padm = nc.dram_tensor("padm", [128, ncols_pad], bf16, kind="ExternalInput")
    b1e = nc.dram_tensor("b1e", [128, 80], f32, kind="ExternalInput")
    b2e = nc.dram_tensor("b2e", [128, 4], bf16, kind="ExternalInput")

    table1 = nc.dram_tensor("table1", [T1ROWS, T1W], bf16, kind="Internal")
    t2shard = nc.dram_tensor("t2shard", [PT // 4, T2W], bf16, kind="Internal")
    table2 = nc.dram_tensor("table2", [T2ROWS, T2W], bf16, kind="Internal",
                            addr_space="Shared")
    outp = nc.dram_tensor("outp", [PT, OUT], f32, kind="ExternalOutput")

    with tile.TileContext(nc) as tc:
        with (
            tc.tile_pool(name="const", bufs=1) as cpool,
            tc.tile_pool(name="node", bufs=3) as npool,
            tc.tile_pool(name="npsum", bufs=2, space="PSUM") as npsum,
            tc.tile_pool(name="gth", bufs=4) as gpool,
            tc.tile_pool(name="edge", bufs=3) as epool,
            tc.tile_pool(name="accs", bufs=2) as apool,
            tc.tile_pool(name="fin", bufs=2) as fpool,
            tc.tile_pool(name="fpsum", bufs=2, space="PSUM") as fpsum,
        ):
            ident = cpool.tile([128, 128], bf16)
            make_identity(nc, ident[:])
            identf = cpool.tile([128, 128], f32)
            make_identity(nc, identf[:])
            w1es = cpool.tile([IN, 80], f32)
            nc.sync.dma_start(out=w1es[:], in_=w1e[:])
            w2es = cpool.tile([HID, 4], bf16)
            nc.sync.dma_start(out=w2es[:], in_=w2e[:])
            b1es = cpool.tile([128, 80], f32)
            nc.sync.dma_start(out=b1es[:], in_=b1e[:])
            b2es = cpool.tile([128, 4], bf16)
            nc.sync.dma_start(out=b2es[:], in_=b2e[:])

            # ---- node phase: table1 = [al_src | h+b1 | al_dst] per node, 4-packed
            for xt in range(NPAD // 128):
                xtile = npool.tile([128, IN], f32, tag="xt")
                nc.sync.dma_start(out=xtile[:], in_=x[xt * 128 : (xt + 1) * 128, :])
                xT_ps = npsum.tile([128, 128], f32, tag="xTp")
                nc.tensor.transpose(out=xT_ps[:], in_=xtile[:], identity=identf[:])
                xTs = npool.tile([128, IN], f32, tag="xTs")
                nc.vector.tensor_copy(out=xTs[:], in_=xT_ps[:])
                g1_ps = npsum.tile([128, 80], f32, tag="g1p")
                nc.tensor.matmul(out=g1_ps[:], lhsT=xTs[:], rhs=w1es[:],
                                 start=True, stop=True)
                t1s = npool.tile([128, 80], bf16, tag="t1s")
                nc.vector.tensor_tensor(out=t1s[:], in0=g1_ps[:], in1=b1es[:],
                                        op=OP.add)
                dst_ap = table1[xt * 32 : xt * 32 + 32, 0:320].rearrange(
                    "r (n v) -> r n v", v=80)
                nc.sync.dma_start(out=dst_ap, in_=t1s[:])

            # ---- edge phase runner: per tile-part (<=32 columns)
            PARTC = 32
            KTMAX = PARTC

            def select4(out_ap, gt, j0, kt, voff, nv, oh_t, tag, ew):
                tmp = epool.tile([128, KTMAX, nv], bf16, tag=f"sel{tag}")
                nc.vector.tensor_tensor(
                    out=out_ap,
                    in0=gt[:, j0 : j0 + kt, voff : voff + nv],
                    in1=oh_t[:, j0 : j0 + kt, 0:1].to_broadcast([128, kt, nv]),
                    op=OP.mult)
                for i in range(1, 4):
                    nc.vector.tensor_tensor(
                        out=tmp[:, 0:kt, :],
                        in0=gt[:, j0 : j0 + kt,
                               i * ew + voff : i * ew + voff + nv],
                        in1=oh_t[:, j0 : j0 + kt, i : i + 1].to_broadcast(
                            [128, kt, nv]),
                        op=OP.mult)
                    nc.vector.tensor_tensor(out=out_ap, in0=out_ap,
                                            in1=tmp[:, 0:kt, :], op=OP.add)

            def edge_phase(layer):
                if layer == 1:
                    idxT, ohT, tabT, EW, NV, EWN = idx1, oh1, table1, T1W, 72, 80
                else:
                    idxT, ohT, tabT, EW, NV, EWN = idx2, oh2, table2, T2W, 3, 4
                WROWS = 8 + HID if layer == 1 else 1 + OUT
                NH = H1 if layer == 1 else 1
                NCCH = C1 if layer == 1 else OUT

                ioff = 0
                gq = 0
                nalt = 0
                for t in range(len(K)):
                    kt_full = int(K[t])
                    base = int(col0[t])
                    adt = None
                    acc = None
                    for p0 in range(0, kt_full, PARTC):
                        kt = min(PARTC, kt_full - p0)
                        first = p0 == 0
                        last = p0 + kt >= kt_full
                        gt = gpool.tile([128, KTMAX, EW], bf16, tag=f"gt{layer}")
                        idx_t = epool.tile([128, kt * 8], i16, tag=f"ix{layer}")
                        eng = nc.sync if (nalt % 2 == 0) else nc.scalar
                        nalt += 1
                        eng.dma_start(out=idx_t[:],
                                      in_=idxT[:, ioff : ioff + kt * 8])
                        for c in range(0, kt, CPC):
                            ncc = min(CPC, kt - c)
                            ni = ncc * 128
                            nc.gpsimd.dma_gather(
                                gt[:, c : c + ncc, :], tabT[:],
                                idx_t[:, c * 8 : (c + ncc) * 8], ni, ni, EW,
                                queue_num=gq % 4)
                            gq += 1
                        ioff += kt * 8
                        oh_t = epool.tile([128, KTMAX, 4], bf16, tag=f"oh{layer}")
                        eng.dma_start(out=oh_t[:, 0:kt, :],
                                      in_=ohT[:, base + p0 : base + p0 + kt, :])
                        pm_t = epool.tile([128, KTMAX], bf16, tag=f"pm{layer}")
                        eng.dma_start(out=pm_t[:, 0:kt],
                                      in_=padm[:, base + p0 : base + p0 + kt])

                        V = epool.tile([128, KTMAX, NV], bf16, tag=f"V{layer}")
                        select4(V[:, 0:kt, :], gt, 0, kt, 0, NV, oh_t,
                                f"v{layer}", EWN)
                        if first:
                            adt = epool.tile([128, 1, NH], bf16,
                                             tag=f"adt{layer}")
                            select4(adt[:], gt, 0, 1, NV, NH, oh_t,
                                    f"a{layer}", EWN)

                        eT = epool.tile([128, KTMAX, NH], f32, tag=f"e{layer}")
                        nc.vector.tensor_tensor(
                            out=eT[:, 0:kt, :], in0=V[:, 0:kt, 0:NH],
                            in1=adt[:].to_broadcast([128, kt, NH]),
                            op=OP.add)
                        nc.vector.tensor_tensor(
                            out=eT[:, 0:kt, :], in0=eT[:, 0:kt, :],
                            in1=pm_t[:, 0:kt].unsqueeze(2).to_broadcast(
                                [128, kt, NH]),
                            op=OP.add)
                        lk = epool.tile([128, KTMAX, NH], f32, tag=f"lk{layer}")
                        nc.vector.tensor_scalar(out=lk[:, 0:kt, :],
                                                in0=eT[:, 0:kt, :], scalar1=NEG,
                                                scalar2=None, op0=OP.mult)
                        nc.vector.tensor_tensor(out=lk[:, 0:kt, :],
                                                in0=lk[:, 0:kt, :],
                                                in1=eT[:, 0:kt, :], op=OP.max)
                        W = epool.tile([128, WROWS, KTMAX], bf16,
                                       tag=f"W{layer}")
                        nc.scalar.activation(
                            out=W[:, 0:NH, 0:kt].rearrange("p h c -> p c h"),
                            in_=lk[:, 0:kt, :], func=ACT.Exp)
                        nc.vector.tensor_tensor(
                            out=W[:, NH:WROWS, 0:kt].rearrange(
                                "p (h c) j -> p h c j", h=NH),
                            in0=V[:, 0:kt, NH : NH + NH * NCCH].rearrange(
                                "p j (h c) -> p h c j", h=NH),
                            in1=W[:, 0:NH, 0:kt].unsqueeze(2).to_broadcast(
                                [128, NH, NCCH, kt]),
                            op=OP.mult)
                        if first:
                            acc = apool.tile([128, WROWS], f32,
                                             tag=f"acc{layer}")
                            nc.vector.tensor_reduce(
                                out=acc[:], in_=W[:, :, 0:kt], axis=AX,
                                op=OP.add)
                        else:
                            red = apool.tile([128, WROWS], f32,
                                             tag=f"red{layer}")
                            nc.vector.tensor_reduce(
                                out=red[:], in_=W[:, :, 0:kt], axis=AX,
                                op=OP.add)
                            nc.vector.tensor_tensor(out=acc[:], in0=acc[:],
                                                    in1=red[:], op=OP.add)
                        if last:
                            finalize(layer, t, acc)

            def finalize(layer, t, a):
                NH = H1 if layer == 1 else 1
                NCCH = C1 if layer == 1 else OUT
                WROWS = 8 + HID if layer == 1 else 1 + OUT
                rden = fpool.tile([128, NH], f32, tag="rden")
                nc.vector.reciprocal(out=rden[:], in_=a[:, 0:NH])
                if layer == 1:
                    z = fpool.tile([128, HID], f32, tag="z")
                    nc.vector.tensor_tensor(
                        out=z[:].rearrange("p (h c) -> p h c", h=NH),
                        in0=a[:, NH:WROWS].rearrange("p (h c) -> p h c", h=NH),
                        in1=rden[:].unsqueeze(2).to_broadcast([128, NH, NCCH]),
                        op=OP.mult)
                    # elu -> bf16
                    zm = fpool.tile([128, HID], f32, tag="zm")
                    nc.vector.tensor_scalar(out=zm[:], in0=z[:], scalar1=0.0,
                                            scalar2=None, op0=OP.min)
                    ze = fpool.tile([128, HID], f32, tag="ze")
                    nc.scalar.activation(out=ze[:], in_=zm[:], func=ACT.Exp)
                    nc.vector.tensor_scalar(out=ze[:], in0=ze[:], scalar1=-1.0,
                                            scalar2=None, op0=OP.add)
                    nc.vector.tensor_scalar(out=zm[:], in0=z[:], scalar1=0.0,
                                            scalar2=None, op0=OP.max)
                    zb = fpool.tile([128, HID], bf16, tag="zb")
                    nc.vector.tensor_tensor(out=zb[:], in0=zm[:], in1=ze[:],
                                            op=OP.add)
                    # G2 row = z @ W2e + b2e
                    zT_ps = fpsum.tile([HID, 128], bf16, tag="zTp")
                    nc.tensor.transpose(out=zT_ps[:], in_=zb[:], identity=ident[:])
                    zTs = fpool.tile([HID, 128], bf16, tag="zTs")
                    nc.vector.tensor_copy(out=zTs[:], in_=zT_ps[:])
                    g2_ps = fpsum.tile([128, 4], f32, tag="g2p")
                    nc.tensor.matmul(out=g2_ps[:], lhsT=zTs[:], rhs=w2es[:],
                                     start=True, stop=True)
                    g2s = fpool.tile([128, 4], bf16, tag="g2s")
                    nc.vector.tensor_tensor(out=g2s[:], in0=g2_ps[:], in1=b2es[:],
                                            op=OP.add)
                    dst_ap = t2shard[t * 32 : t * 32 + 32, 0:16].rearrange(
                        "r (n v) -> r n v", v=4)
                    nc.sync.dma_start(out=dst_ap, in_=g2s[:])
                else:
                    o = fpool.tile([128, OUT], f32, tag="o2")
                    nc.vector.tensor_tensor(
                        out=o[:], in0=a[:, 1 : 1 + OUT],
                        in1=rden[:].to_broadcast([128, OUT]),
                        op=OP.mult)
                    nc.sync.dma_start(
                        out=outp[t * 128 : (t + 1) * 128, :], in_=o[:])

            if phases in ("e1", "cc", "all"):
                edge_phase(1)

            if phases in ("cc", "all"):
                # ---- exchange layer-2 node features
                nc.gpsimd.collective_compute(
                    "AllGather",
                    OP.bypass,
                    replica_groups=[list(range(NCORES))],
                    ins=[t2shard[:]],
                    outs=[table2[:]],
                )

            if phases == "all":
                edge_phase(2)

    nc.compile()
    return nc


def kernel(**inputs):
    from concourse.bass_utils import run_bass_kernel_spmd

    x = np.asarray(inputs["x"], dtype=np.float32)
    ei = np.asarray(inputs["edge_index"]).astype(np.int64)
    w1 = np.asarray(inputs["W1"], dtype=np.float32)
    a1s = np.asarray(inputs["a1_src"], dtype=np.float32)
    a1d = np.asarray(inputs["a1_dst"], dtype=np.float32)
    b1 = np.asarray(inputs["b1"], dtype=np.float32)
    w2 = np.asarray(inputs["W2"], dtype=np.float32)
    a2s = np.asarray(inputs["a2_src"], dtype=np.float32)
    a2d = np.asarray(inputs["a2_dst"], dtype=np.float32)
    b2 = np.asarray(inputs["b2"], dtype=np.float32)

    src = ei[0]
    dst = ei[1]

    datas, K, col0, ctile, cj, ncalls, ncols_pad = _plan(src, dst)
    chunks = []
    for t in range(NT):
        for c in range(0, int(K[t]), CPC):
            chunks.append((int(col0[t]) + c, min(CPC, int(K[t]) - c)))
    totc = sum(nc_ * 8 for _, nc_ in chunks)

    # permuted global position of each node for the L2 table
    gpos_of_node = np.zeros(NPAD, dtype=np.int64)
    for c in range(NCORES):
        perm = datas[c]["perm"]  # [PT] local dst ids (or -1)
        real = perm >= 0
        gpos_of_node[c * ND + perm[real]] = c * PT + np.nonzero(real)[0]

    per_core = _pack_inputs(datas, gpos_of_node, chunks, ncols_pad)

    # weights
    A1s = np.zeros((HID, H1), dtype=np.float32)
    A1d = np.zeros((HID, H1), dtype=np.float32)
    for h in range(H1):
        A1s[h * C1 : (h + 1) * C1, h] = a1s[h]
        A1d[h * C1 : (h + 1) * C1, h] = a1d[h]
    w1e = np.concatenate([w1 @ A1s, w1, w1 @ A1d], axis=1)      # [128, 80]
    w2e = np.concatenate([w2 @ a2s.T, w2, w2 @ a2d.T], axis=1)  # [64, 4]
    b1e = np.zeros((128, 80), dtype=np.float32)
    b1e[:, 8 : 8 + HID] = b1[None, :]
    b2e = np.zeros((128, 4), dtype=np.float32)
    b2e[:, 1 : 1 + OUT] = b2[None, :]

    xp = np.zeros((NPAD, IN), dtype=np.float32)
    xp[:N] = x

    key = (ncalls, tuple(K.tolist()))
    if key not in _BUILD_CACHE:
        _BUILD_CACHE[key] = _build(K, col0, chunks, totc, ncols_pad)
    nc = _BUILD_CACHE[key]

    common = dict(x=xp, w1e=w1e, w2e=w2e.astype(BF16), b1e=b1e,
                  b2e=b2e.astype(BF16))
    in_maps = []
    for c in range(NCORES):
        m = dict(common)
        m.update(per_core[c])
        in_maps.append(m)

    global _LAST_IN_MAPS
    _LAST_IN_MAPS = in_maps
    res = run_bass_kernel_spmd(nc, in_maps, list(range(NCORES)))

    out = np.zeros((N, OUT), dtype=np.float32)
    for c in range(NCORES):
        op = res.results[c]["outp"]       # [PT, 2] in permuted order
        perm = datas[c]["perm"]
        real = perm >= 0
        out[c * ND + perm[real]] = op[real]
    return out

